# revision 1
# baseline (speedup 1.0000x reference)
"""Trainium2 Bass kernel for nn_MultiHeadAttention (B=4,T=2048,C=1024,H=16,D=64).

Sharding: tensor-parallel over heads. 8 cores x 2 heads each.
Per core: QKV column slices (128 dims), full attention for its 2 heads,
Wo row slice -> partial output summed on host.

Device layout (per core, per batch b):
  qt/kt [128, T]  : rows = 2 heads x 64 dims, cols = seq. f32r.
  vsb   [128, NKT, 130] : per key-tile [V_h0 | ones | V_h1 | ones] (65 each).
  S.T   psum [128 keys, 512 q] -> exp (ACT, scale 1/8) -> f32r SBUF
  PV    : lhsT = V_aug [128, 65] -> psum [65, 512]: rows 0:64 = O.T, row 64 = sumexp
  norm scales + 1/sumexp broadcast across partitions via DRAM-roundtrip DMA.
  Wo    : lhsT = yT [128, 128t] @ WoT [128, 512] -> partial out rows.
All matmuls fp32r (inputs pre-rounded to 11-bit mantissa on host or by engines).
"""
import sys

sys.path.insert(0, "/opt/trn_rl_repo")
import numpy as np

N_CORES = 8
B_FULL, T_FULL, C = 4, 2048, 1024
H, D = 16, 64
HPC = H // N_CORES          # heads per core = 2
M2 = HPC * D                # 128
EPS = 1e-6

_NC_CACHE: dict = {}


def round_fp32r(x: np.ndarray) -> np.ndarray:
    u = np.ascontiguousarray(x, dtype=np.float32).view(np.uint32)
    r = (u + 0x800 + ((u >> 12) & 1)) & 0xFFFFF000
    return r.view(np.float32)


def build_nc(B: int, T: int, debug_taps: bool = False):
    import concourse.bass as bass
    import concourse.mybir as mybir
    from concourse import bacc
    from concourse.tile import TileContext

    F32R = mybir.dt.float32r
    F32 = mybir.dt.float32
    AF = mybir.ActivationFunctionType

    TT = B * T
    NCIN = C // 128             # 8 contraction tiles for projections
    CPB = T // 512              # 512-col chunks per batch
    NKT = T // 128              # key tiles per batch
    KGS = 2                     # key tiles per exp group (PSUM-bank limited)
    NKG = NKT // KGS
    NQC = T // 512              # q chunks per batch

    nc = bacc.Bacc("TRN2", target_bir_lowering=False, debug=False,
                   num_devices=N_CORES)

    xT_d = nc.dram_tensor("xT", [C, TT], F32R, kind="ExternalInput")
    wq_d = nc.dram_tensor("wq", [C, M2], F32R, kind="ExternalInput")
    wk_d = nc.dram_tensor("wk", [C, M2], F32R, kind="ExternalInput")
    wv_d = nc.dram_tensor("wv", [C, M2], F32R, kind="ExternalInput")
    wo_d = nc.dram_tensor("wo", [M2, C], F32R, kind="ExternalInput")
    cos_d = nc.dram_tensor("cos2", [M2, T], F32, kind="ExternalInput")
    sin_d = nc.dram_tensor("sin2s", [M2, T], F32, kind="ExternalInput")
    ident_d = nc.dram_tensor("ident", [128, 128], F32R, kind="ExternalInput")
    ones2_d = nc.dram_tensor("ones2c", [128, 2], F32R, kind="ExternalInput")
    onesk_d = nc.dram_tensor("onesk", [128, NKT], F32R, kind="ExternalInput")
    out_d = nc.dram_tensor("out", [TT, C], F32, kind="ExternalOutput")
    taps = {}
    if debug_taps:
        taps["qt"] = nc.dram_tensor("tap_qt", [128, T], F32, kind="ExternalOutput")
        taps["kt"] = nc.dram_tensor("tap_kt", [128, T], F32, kind="ExternalOutput")
        taps["vsb"] = nc.dram_tensor("tap_vsb", [128, NKT, 130], F32, kind="ExternalOutput")
        taps["scales"] = nc.dram_tensor("tap_scales", [128, 4 * T // 128], F32, kind="ExternalOutput")
        taps["st"] = nc.dram_tensor("tap_st", [128, KGS, 512], F32, kind="ExternalOutput")
        taps["ot"] = nc.dram_tensor("tap_ot", [65, 512], F32, kind="ExternalOutput")
        taps["yt"] = nc.dram_tensor("tap_yt", [128, T], F32, kind="ExternalOutput")

    def bcast_rows(tile_ap, row_elem_offset, n_rows, n_cols):
        # DRAM source AP broadcasting one row across n_rows partitions
        return bass.AP(tensor=tile_ap.tensor,
                       offset=tile_ap.offset + row_elem_offset,
                       ap=[[0, n_rows], [1, n_cols]])

    with TileContext(nc) as tc:
        with (
            tc.tile_pool(name="const", bufs=1) as cp,
            tc.tile_pool(name="big", bufs=2) as bigp,
            tc.tile_pool(name="xs", bufs=2) as xsp,
            tc.tile_pool(name="attn", bufs=(2 if debug_taps else 3)) as atp,
            tc.tile_pool(name="scr", bufs=2) as scp,
            tc.tile_pool(name="drs", bufs=2, space="DRAM") as drp,
            tc.tile_pool(name="ps", bufs=1, space="PSUM") as psp,
        ):
            wq_sb = cp.tile([128, NCIN, M2], F32R, tag="wq")
            wk_sb = cp.tile([128, NCIN, M2], F32R, tag="wk")
            wv_sb = cp.tile([128, NCIN, M2], F32R, tag="wv")
            wo_sb = cp.tile([128, C], F32R, tag="wo")
            cos_sb = cp.tile([128, T], F32, tag="cos")
            sin_sb = cp.tile([128, T], F32, tag="sin")
            ident = cp.tile([128, 128], F32R, tag="ident")
            ones2 = cp.tile([128, 2], F32R, tag="ones2")

            for w_sb, w_d in ((wq_sb, wq_d), (wk_sb, wk_d), (wv_sb, wv_d)):
                nc.sync.dma_start(
                    out=w_sb, in_=w_d.rearrange("(co ci) m -> ci co m", ci=128))
            nc.sync.dma_start(out=wo_sb, in_=wo_d[:, :])
            nc.sync.dma_start(out=cos_sb, in_=cos_d[:, :])
            nc.sync.dma_start(out=sin_sb, in_=sin_d[:, :])
            nc.sync.dma_start(out=ident, in_=ident_d[:, :])
            nc.sync.dma_start(out=ones2, in_=ones2_d[:, :])

            ROPE_SLICES = (((0, 32), (32, 64)), ((32, 64), (0, 32)),
                           ((64, 96), (96, 128)), ((96, 128), (64, 96)))

            for b in range(B):
                qtb = bigp.tile([128, T], F32R, tag="qtb")
                ktb = bigp.tile([128, T], F32R, tag="ktb")
                ytb = bigp.tile([128, T], F32R, tag="ytb")
                vsb = bigp.tile([128, NKT, 130], F32R, tag="vsb")
                ssq = scp.tile([2, T], F32, tag="ssq")
                ssk = scp.tile([2, T], F32, tag="ssk")
                sums_t = drp.tile([4, T], F32, tag="sums")
                scales_t = drp.tile([4, T], F32, tag="scales")
                recip_t = drp.tile([2, T], F32, tag="recip")
                nc.sync.dma_start(out=vsb[:, :, 64:65], in_=onesk_d[:, :, None])
                nc.sync.dma_start(out=vsb[:, :, 129:130], in_=onesk_d[:, :, None])
                qtb_f = qtb[:].bitcast(F32)
                ktb_f = ktb[:].bitcast(F32)
                ytb_f = ytb[:].bitcast(F32)

                # ---- phase 1: projections + RoPE + sumsq + V transpose ----
                for c in range(CPB):
                    cc = slice(c * 512, (c + 1) * 512)
                    x_sb = xsp.tile([128, NCIN, 512], F32R, tag="x")
                    nc.sync.dma_start(
                        out=x_sb,
                        in_=xT_d[:, b * T + c * 512: b * T + (c + 1) * 512]
                        .rearrange("(co ci) t -> ci co t", ci=128))

                    for name, w_sb, dst, dst_f, ss_sb in (
                            ("q", wq_sb, qtb, qtb_f, ssq),
                            ("k", wk_sb, ktb, ktb_f, ssk)):
                        ps = psp.tile([128, 512], F32, tag="mm512", bufs=2)
                        for ci in range(NCIN):
                            nc.tensor.matmul(ps, w_sb[:, ci], x_sb[:, ci],
                                             start=(ci == 0), stop=(ci == NCIN - 1),
                                             skip_group_check=True)
                        # RoPE: dst = ps*cos + rot_half(ps)*sin_signed
                        # (written as f32r: every writer of qtb/ktb bytes must
                        #  have f32r output dtype for the fp32r matmul verifier)
                        dd = dst[:, cc]
                        dd_f = dst_f[:, cc]
                        nc.vector.tensor_mul(out=dd, in0=ps, in1=cos_sb[:, cc])
                        rot = scp.tile([128, 512], F32, tag="rot")
                        for (d0, d1), (s0, s1) in ROPE_SLICES:
                            nc.vector.tensor_mul(
                                out=rot[d0:d1], in0=ps[s0:s1],
                                in1=sin_sb[d0:d1, cc])
                        nc.vector.tensor_add(out=dd, in0=dd_f, in1=rot)
                        # sum of squares over d (per head) via ones-matmul
                        sq = scp.tile([128, 512], F32R, tag="sq")
                        nc.vector.tensor_mul(out=sq, in0=dd_f, in1=dd_f)
                        ps_ss = psp.tile([2, 512], F32, tag="psmall", bufs=2)
                        nc.tensor.matmul(ps_ss, ones2, sq, start=True, stop=True,
                                         skip_group_check=True)
                        nc.vector.tensor_copy(out=ss_sb[:, cc], in_=ps_ss)

                    # V: project then transpose to [t, d] layout
                    ps = psp.tile([128, 512], F32, tag="mm512", bufs=2)
                    for ci in range(NCIN):
                        nc.tensor.matmul(ps, wv_sb[:, ci], x_sb[:, ci],
                                         start=(ci == 0), stop=(ci == NCIN - 1),
                                         skip_group_check=True)
                    vtmp = scp.tile([128, 512], F32R, tag="vtmp")
                    nc.vector.tensor_copy(out=vtmp, in_=ps)
                    for i in range(4):
                        kt_idx = c * 4 + i
                        ps_t = psp.tile([128, 128], F32R, tag="psmall", bufs=2)
                        nc.tensor.transpose(ps_t, vtmp[:, i * 128:(i + 1) * 128],
                                            ident)
                        nc.vector.tensor_copy(out=vsb[:, kt_idx, 0:64],
                                              in_=ps_t[:, 0:64])
                        nc.vector.tensor_copy(out=vsb[:, kt_idx, 65:129],
                                              in_=ps_t[:, 64:128])

                # ---- norm scales: rsqrt via recip+sqrt+Newton on packed tile ----
                nc.sync.dma_start(out=sums_t[0:2, :], in_=ssq)
                nc.sync.dma_start(out=sums_t[2:4, :], in_=ssk)
                FP = 4 * T // 128  # packed free dim
                pk = scp.tile([128, 6, FP], F32, tag="pk")
                nc.sync.dma_start(
                    out=pk[:, 0],
                    in_=sums_t[:].rearrange("a t -> (a t)")
                    .rearrange("(p f) -> p f", p=128))
                ms, r0, y0, t1, y1 = (pk[:, j] for j in range(1, 6))
                nc.vector.tensor_scalar(out=ms, in0=pk[:, 0], scalar1=1.0 / D,
                                        scalar2=EPS, op0=mybir.AluOpType.mult,
                                        op1=mybir.AluOpType.add)
                nc.vector.reciprocal_approx_accurate(out=r0, in_=ms, scratch=y0)
                nc.scalar.activation(y0, r0, AF.Sqrt)
                # one Newton step: y1 = y0 * (1.5 - 0.5 * ms * y0^2)
                nc.vector.tensor_mul(out=t1, in0=ms, in1=y0)
                nc.vector.tensor_mul(out=t1, in0=t1, in1=y0)
                nc.vector.tensor_scalar(out=t1, in0=t1, scalar1=-0.5,
                                        scalar2=1.5, op0=mybir.AluOpType.mult,
                                        op1=mybir.AluOpType.add)
                nc.vector.tensor_mul(out=y1, in0=y0, in1=t1)
                nc.sync.dma_start(
                    out=scales_t[:].rearrange("a t -> (a t)")
                    .rearrange("(p f) -> p f", p=128),
                    in_=y1)
                if debug_taps and b == 0:
                    nc.sync.dma_start(out=taps["scales"][:, :], in_=y1)
                for c in range(CPB):
                    cc = slice(c * 512, (c + 1) * 512)
                    for ti, (tile, tile_f) in enumerate(((qtb, qtb_f), (ktb, ktb_f))):
                        brc = scp.tile([128, 512], F32, tag="brc")
                        for hh in range(2):
                            off = (ti * 2 + hh) * T + c * 512
                            nc.sync.dma_start(
                                out=brc[hh * 64:(hh + 1) * 64],
                                in_=bcast_rows(scales_t[:], off, 64, 512))
                        nc.vector.tensor_mul(out=tile[:, cc], in0=tile_f[:, cc],
                                             in1=brc)

                if debug_taps and b == 0:
                    nc.sync.dma_start(out=taps["qt"][:, :], in_=qtb[:].bitcast(F32))
                    nc.sync.dma_start(out=taps["kt"][:, :], in_=ktb[:].bitcast(F32))
                    nc.sync.dma_start(out=taps["vsb"][:, :, :], in_=vsb[:].bitcast(F32))

                # ---- phase 2: attention ----
                for qc in range(NQC):
                    qq = slice(qc * 512, (qc + 1) * 512)
                    ot0 = psp.tile([65, 512], F32, tag="ot0", bufs=1)
                    ot1 = psp.tile([65, 512], F32, tag="ot1", bufs=1)
                    ots = [ot0, ot1]
                    for h in range(2):
                        hs = slice(h * 64, (h + 1) * 64)
                        q_rhs = qtb[hs, qq]
                        for kg in range(NKG):
                            sps = psp.tile([128, KGS, 512], F32, tag="sgrp",
                                           bufs=1)
                            for i in range(KGS):
                                ktg = kg * KGS + i
                                nc.tensor.matmul(
                                    sps[:, i],
                                    ktb[hs, ktg * 128:(ktg + 1) * 128], q_rhs,
                                    start=True, stop=True, skip_group_check=True)
                            stexp = atp.tile([128, KGS, 512], F32R, tag="stexp")
                            nc.scalar.activation(stexp, sps, AF.Exp, scale=0.125)
                            if debug_taps and b == 0 and qc == 0 and h == 0 and kg == 0:
                                nc.sync.dma_start(out=taps["st"][:, :, :],
                                                  in_=stexp[:].bitcast(F32))
                            for i in range(KGS):
                                ktg = kg * KGS + i
                                nc.tensor.matmul(
                                    ots[h], vsb[:, ktg, h * 65:h * 65 + 65],
                                    stexp[:, i],
                                    start=(ktg == 0), stop=(ktg == NKT - 1),
                                    skip_group_check=True)
                    if debug_taps and b == 0 and qc == 0:
                        otc = scp.tile([65, 512], F32, tag="otc", bufs=1)
                        nc.vector.tensor_copy(out=otc, in_=ots[0][:, :])
                        nc.sync.dma_start(out=taps["ot"][:, :], in_=otc)
                    # reciprocal of sumexp (row 64) -> dram for broadcast
                    for h in range(2):
                        rcp = scp.tile([128, 512], F32, tag="rcp")
                        nc.vector.reciprocal(rcp[64:65, 0:512], ots[h][64:65])
                        nc.sync.dma_start(out=recip_t[h:h + 1, qq],
                                          in_=rcp[64:65, 0:512])
                    # move O rows into ytb (h0 aligned; h1 via DMA shift)
                    nc.vector.tensor_copy(out=ytb[0:64, qq], in_=ots[0][0:64])
                    stg = scp.tile([64, 512], F32R, tag="stg")
                    nc.vector.tensor_copy(out=stg, in_=ots[1][0:64])
                    nc.sync.dma_start(out=ytb[64:128, qq], in_=stg)
                    # normalize: ytb *= 1/sumexp (broadcast from dram)
                    brc = scp.tile([128, 512], F32, tag="brc")
                    for h in range(2):
                        off = h * T + qc * 512
                        nc.sync.dma_start(
                            out=brc[h * 64:(h + 1) * 64],
                            in_=bcast_rows(recip_t[:], off, 64, 512))
                    nc.vector.tensor_mul(out=ytb[:, qq], in0=ytb_f[:, qq], in1=brc)

                if debug_taps and b == 0:
                    nc.sync.dma_start(out=taps["yt"][:, :], in_=ytb[:].bitcast(F32))

                # ---- phase 3: Wo projection, partial output ----
                for tt in range(T // 128):
                    for oc in range(C // 512):
                        pso = psp.tile([128, 512], F32, tag="mm512", bufs=2)
                        nc.tensor.matmul(pso, ytb[:, tt * 128:(tt + 1) * 128],
                                         wo_sb[:, oc * 512:(oc + 1) * 512],
                                         start=True, stop=True,
                                         skip_group_check=True)
                        ob = scp.tile([128, 512], F32, tag="ob")
                        nc.vector.tensor_copy(out=ob, in_=pso)
                        nc.sync.dma_start(
                            out=out_d[b * T + tt * 128: b * T + (tt + 1) * 128,
                                      oc * 512:(oc + 1) * 512],
                            in_=ob)

    nc.compile()
    return nc


def make_core_inputs(x, cos, sin, Wq, Wk, Wv, Wo, B, T):
    """Host-side sharding. Returns list of 8 input dicts."""
    TT = B * T
    xT = np.ascontiguousarray(round_fp32r(np.asarray(x, np.float32).reshape(TT, C)).T)
    cosT = np.asarray(cos, np.float32).reshape(T, D).T      # [64, T]
    sinT = np.asarray(sin, np.float32).reshape(T, D).T
    sin_signed = np.concatenate([-sinT[0:32], sinT[32:64]], axis=0)
    cos2 = np.ascontiguousarray(np.concatenate([cosT, cosT], axis=0))
    sin2 = np.ascontiguousarray(np.concatenate([sin_signed, sin_signed], axis=0))
    ones2c = np.zeros((128, 2), np.float32)
    ones2c[0:64, 0] = 1.0
    ones2c[64:128, 1] = 1.0
    in_maps = []
    for core in range(N_CORES):
        rows = slice(core * M2, (core + 1) * M2)
        in_maps.append({
            "xT": xT,
            "wq": np.ascontiguousarray(round_fp32r(np.asarray(Wq, np.float32)[rows]).T),
            "wk": np.ascontiguousarray(round_fp32r(np.asarray(Wk, np.float32)[rows]).T),
            "wv": np.ascontiguousarray(round_fp32r(np.asarray(Wv, np.float32)[rows]).T),
            "wo": np.ascontiguousarray(round_fp32r(np.asarray(Wo, np.float32))[:, rows].T),
            "cos2": cos2,
            "sin2s": sin2,
            "ident": np.eye(128, dtype=np.float32),
            "ones2c": ones2c,
            "onesk": np.ones((128, T // 128), np.float32),
        })
    return in_maps


def kernel(x, cos, sin, Wq, Wk, Wv, Wo):
    from concourse.bass_utils import run_bass_kernel_spmd

    B, T = x.shape[0], x.shape[1]
    key = (B, T)
    if key not in _NC_CACHE:
        _NC_CACHE[key] = build_nc(B, T)
    nc = _NC_CACHE[key]
    in_maps = make_core_inputs(x, cos, sin, Wq, Wk, Wv, Wo, B, T)
    res = run_bass_kernel_spmd(nc, in_maps, core_ids=list(range(N_CORES)))
    out = np.zeros((B * T, C), np.float64)
    for r in res.results:
        out += r["out"].astype(np.float64)
    return out.astype(np.float32).reshape(B, T, C)



# revision 14
# speedup vs baseline: 1.9812x; 1.9812x over previous
"""Trainium2 Bass kernel for nn_MultiHeadAttention (B=4,T=2048,C=1024,H=16,D=64).

Sharding: batch x head-group. 8 cores = 4 batches x 2 groups of 8 heads.
Per core: one batch's x (loaded once, bf16), QKV column slices for its 8
heads (4 head-pairs), full attention, Wo row slice -> partial [T, C] output;
host sums the 2 partials per batch.

Per head-pair hp (2 heads on 128 partitions):
  qtb/ktb [128, T] bf16 : rows = 2 heads x 64 dims, cols = seq.
  vsb    [128, NKT, 130] bf16 : per key-tile [V_h0 | ones | V_h1 | ones].
  scores: two row-tiled K=64 matmuls (h0 rows 0-63, h1 rows 64-127) ->
          psum [128, 2, 512]; ACT exp (scale 1/8) -> stexp bf16.
  PV: lhsT = V_aug [128, 65] -> psum [65, 512]: rows 0:64 = O.T, row 64 =
      sumexp. 1/sumexp via packed reciprocal + DRAM broadcast.
  RMS norm scales via DVE fast-inverse-sqrt (bit trick + 2 Newton steps).
Wo phase at the end accumulates all 4 head-pairs per output tile.
"""
import sys

sys.path.insert(0, "/opt/trn_rl_repo")
import numpy as np
import ml_dtypes

N_CORES = 8
B_FULL, T_FULL, C = 4, 2048, 1024
H, D = 16, 64
HPC = H // 2                 # heads per core = 8
NHP = HPC // 2               # head-pairs per core = 4
M2 = 2 * D                   # 128 dims per head-pair
MG = HPC * D                 # 512 dims per core (head-group)
EPS = 1e-6
MAGIC = 0x5F3759DF

_NC_CACHE: dict = {}

BF16 = ml_dtypes.bfloat16


def build_nc(T: int, debug_taps: bool = False):
    import concourse.bass as bass
    import concourse.mybir as mybir
    from concourse import bacc
    from concourse.tile import TileContext

    F32 = mybir.dt.float32
    BF = mybir.dt.bfloat16
    I32 = mybir.dt.int32
    AF = mybir.ActivationFunctionType
    OP = mybir.AluOpType

    NCIN = C // 128              # 8 contraction tiles for projections
    CPB = T // 512               # 4 x 512-col chunks of T
    NKT = T // 128               # 16 key tiles
    NQC = T // 512               # 4 q chunks

    nc = bacc.Bacc("TRN2", target_bir_lowering=False, debug=False,
                   num_devices=N_CORES)

    xT_d = nc.dram_tensor("xT", [C, T], BF, kind="ExternalInput")
    wq_d = nc.dram_tensor("wq", [C, MG], BF, kind="ExternalInput")
    wk_d = nc.dram_tensor("wk", [C, MG], BF, kind="ExternalInput")
    wv_d = nc.dram_tensor("wv", [C, MG], BF, kind="ExternalInput")
    wo_d = nc.dram_tensor("wo", [MG, C], BF, kind="ExternalInput")
    cos_d = nc.dram_tensor("cos2", [128, T], BF, kind="ExternalInput")
    sin_d = nc.dram_tensor("sin2s", [128, T], BF, kind="ExternalInput")
    ident_d = nc.dram_tensor("ident", [128, 128], BF, kind="ExternalInput")
    ones2_d = nc.dram_tensor("ones2c", [128, 2], BF, kind="ExternalInput")
    onesk_d = nc.dram_tensor("onesk", [128, NKT], BF, kind="ExternalInput")
    out_d = nc.dram_tensor("out", [T, C], F32, kind="ExternalOutput")
    taps = {}
    if debug_taps:
        taps["qt"] = nc.dram_tensor("tap_qt", [128, T], BF, kind="ExternalOutput")
        taps["kt"] = nc.dram_tensor("tap_kt", [128, T], BF, kind="ExternalOutput")
        taps["vsb"] = nc.dram_tensor("tap_vsb", [128, NKT, 130], BF, kind="ExternalOutput")
        taps["scales"] = nc.dram_tensor("tap_scales", [128, 64], F32, kind="ExternalOutput")
        taps["st"] = nc.dram_tensor("tap_st", [128, 2, 512], BF, kind="ExternalOutput")
        taps["ot"] = nc.dram_tensor("tap_ot", [65, 512], F32, kind="ExternalOutput")
        taps["yt"] = nc.dram_tensor("tap_yt", [128, 4, T], BF, kind="ExternalOutput")

    def bcast_rows(tile_ap, row_elem_offset, n_rows, n_cols):
        # DRAM source AP broadcasting one row across n_rows partitions
        return bass.AP(tensor=tile_ap.tensor,
                       offset=tile_ap.offset + row_elem_offset,
                       ap=[[0, n_rows], [1, n_cols]])

    ROPE_SLICES = (((0, 32), (32, 64)), ((32, 64), (0, 32)),
                   ((64, 96), (96, 128)), ((96, 128), (64, 96)))

    with TileContext(nc) as tc:
        with (
            tc.tile_pool(name="const", bufs=1) as cp,
            tc.tile_pool(name="big", bufs=2) as bigp,
            tc.tile_pool(name="attn", bufs=3) as atp,
            tc.tile_pool(name="scr", bufs=2) as scp,
            tc.tile_pool(name="drs", bufs=2, space="DRAM") as drp,
            tc.tile_pool(name="ps", bufs=1, space="PSUM") as psp,
        ):
            x_sb = cp.tile([128, NCIN, T], BF, tag="x")
            wq_sb = cp.tile([128, NCIN, NHP, 128], BF, tag="wq")
            wk_sb = cp.tile([128, NCIN, NHP, 128], BF, tag="wk")
            wv_sb = cp.tile([128, NCIN, NHP, 128], BF, tag="wv")
            wo_sb = cp.tile([128, NHP, C], BF, tag="wo")
            cos_sb = cp.tile([128, T], BF, tag="cos")
            sin_sb = cp.tile([128, T], BF, tag="sin")
            ident = cp.tile([128, 128], BF, tag="ident")
            ones2 = cp.tile([128, 2], BF, tag="ones2")
            ytb_all = cp.tile([128, NHP, T], BF, tag="ytb")

            for cc4 in range(CPB):
                sl = slice(cc4 * 512, (cc4 + 1) * 512)
                nc.sync.dma_start(
                    out=x_sb[:, :, sl],
                    in_=xT_d[:, sl].rearrange("(co ci) t -> ci co t", ci=128))
            for w_sb, w_d in ((wq_sb, wq_d), (wk_sb, wk_d), (wv_sb, wv_d)):
                nc.sync.dma_start(
                    out=w_sb,
                    in_=w_d.rearrange("(co ci) (hp m) -> ci co hp m",
                                      ci=128, m=128))
            nc.sync.dma_start(
                out=wo_sb, in_=wo_d.rearrange("(hp ci) o -> ci hp o", ci=128))
            nc.sync.dma_start(out=cos_sb, in_=cos_d[:, :])
            nc.sync.dma_start(out=sin_sb, in_=sin_d[:, :])
            nc.sync.dma_start(out=ident, in_=ident_d[:, :])
            nc.sync.dma_start(out=ones2, in_=ones2_d[:, :])

            for hp in range(NHP):
                qtb = bigp.tile([128, T], BF, tag="qtb")
                ktb = bigp.tile([128, T], BF, tag="ktb")
                vsb = bigp.tile([128, NKT, 130], BF, tag="vsb")
                ss_q = bigp.tile([2, T], F32, tag="ssq", bufs=1)
                ss_k = bigp.tile([2, T], F32, tag="ssk", bufs=1)
                sums_t = drp.tile([4, T], F32, tag="sums")
                scales_t = drp.tile([4, T], F32, tag="scales")
                se_t = drp.tile([2, T], F32, tag="se")
                rcp_t = drp.tile([2, T], F32, tag="rcp")
                nc.sync.dma_start(out=vsb[:, :, 64:65], in_=onesk_d[:, :, None])
                nc.sync.dma_start(out=vsb[:, :, 129:130], in_=onesk_d[:, :, None])

                # ---- phase 1: projections + RoPE + sumsq + V transpose ----
                for c in range(CPB):
                    cc = slice(c * 512, (c + 1) * 512)
                    for name, w_sb, dst, ss_sb in (
                            ("q", wq_sb, qtb, ss_q),
                            ("k", wk_sb, ktb, ss_k)):
                        ps = psp.tile([128, 512], F32, tag="mm", bufs=2)
                        for ci in range(NCIN):
                            nc.tensor.matmul(ps, w_sb[:, ci, hp],
                                             x_sb[:, ci, cc],
                                             start=(ci == 0),
                                             stop=(ci == NCIN - 1),
                                             skip_group_check=True)
                        pcp = scp.tile([128, 512], BF, tag="pcp")
                        nc.scalar.copy(out=pcp, in_=ps)
                        # RoPE: dst = pcp*cos + rot_half(pcp)*sin_signed.
                        # sin2s is host-permuted to SOURCE-row order so both
                        # SBUF inputs share a base partition; only the output
                        # AP carries the rotate-half partition shift.
                        rot = scp.tile([128, 512], BF, tag="rot")
                        for (d0, d1), (s0, s1) in ROPE_SLICES:
                            nc.vector.tensor_mul(
                                out=rot[d0:d1], in0=pcp[s0:s1],
                                in1=sin_sb[s0:s1, cc])
                        nc.vector.tensor_mul(out=dst[:, cc], in0=pcp,
                                             in1=cos_sb[:, cc])
                        nc.vector.tensor_add(out=dst[:, cc], in0=dst[:, cc],
                                             in1=rot)
                        # sum of squares over d (per head) via ones-matmul
                        sq = scp.tile([128, 512], BF, tag="sq")
                        nc.gpsimd.tensor_mul(out=sq, in0=dst[:, cc],
                                             in1=dst[:, cc])
                        ps_ss = psp.tile([2, 512], F32, tag="mm", bufs=2)
                        nc.tensor.matmul(ps_ss, ones2, sq, start=True,
                                         stop=True, skip_group_check=True)
                        nc.vector.tensor_copy(out=ss_sb[:, cc], in_=ps_ss)

                    # V: project then transpose to [t, d] layout
                    ps = psp.tile([128, 512], F32, tag="mm", bufs=2)
                    for ci in range(NCIN):
                        nc.tensor.matmul(ps, wv_sb[:, ci, hp], x_sb[:, ci, cc],
                                         start=(ci == 0), stop=(ci == NCIN - 1),
                                         skip_group_check=True)
                    vtmp = scp.tile([128, 512], BF, tag="vtmp")
                    nc.scalar.copy(out=vtmp, in_=ps)
                    for i in range(4):
                        kt_idx = c * 4 + i
                        ps_t = psp.tile([128, 128], BF, tag="mm", bufs=2)
                        nc.tensor.transpose(ps_t, vtmp[:, i * 128:(i + 1) * 128],
                                            ident)
                        dst_ap = bass.AP(
                            tensor=vsb.tensor,
                            offset=vsb[:, kt_idx, 0:64].offset,
                            ap=[vsb[:, kt_idx, 0:64].ap[0], [65, 2], [1, 64]])
                        nc.vector.tensor_copy(
                            out=dst_ap,
                            in_=ps_t[:, :].rearrange("p (a b) -> p a b", a=2))

                # ---- norm scales: rsqrt via fast-inverse-sqrt on DVE ----
                nc.sync.dma_start(out=sums_t[0:2, :], in_=ss_q)
                nc.sync.dma_start(out=sums_t[2:4, :], in_=ss_k)
                FP = 4 * T // 128  # 64 packed free dim
                pk = scp.tile([128, 8, FP], F32, tag="pk")
                ms, t1, y1, t2 = (pk[:, j] for j in range(1, 5))
                hi = pk[:, 5].bitcast(I32)
                y0i = pk[:, 6].bitcast(I32)
                nc.sync.dma_start(
                    out=pk[:, 0],
                    in_=sums_t[:].rearrange("a t -> (a t)")
                    .rearrange("(p f) -> p f", p=128))
                nc.vector.tensor_scalar(out=ms, in0=pk[:, 0], scalar1=1.0 / D,
                                        scalar2=EPS, op0=OP.mult, op1=OP.add)
                # y0 = bitcast(MAGIC - (bits(ms) >> 1)); 2 Newton steps
                nc.vector.tensor_scalar(out=hi, in0=ms.bitcast(I32),
                                        scalar1=1, scalar2=None,
                                        op0=OP.logical_shift_right)
                nc.vector.tensor_scalar(out=y0i, in0=hi, scalar1=-1.0,
                                        scalar2=float(MAGIC),
                                        op0=OP.mult, op1=OP.add)
                y0 = pk[:, 6]
                nc.vector.tensor_mul(out=t1, in0=ms, in1=y0)
                nc.vector.tensor_mul(out=t1, in0=t1, in1=y0)
                nc.vector.tensor_scalar(out=t1, in0=t1, scalar1=-0.5,
                                        scalar2=1.5, op0=OP.mult, op1=OP.add)
                nc.vector.tensor_mul(out=y1, in0=y0, in1=t1)
                nc.vector.tensor_mul(out=t2, in0=ms, in1=y1)
                nc.vector.tensor_mul(out=t2, in0=t2, in1=y1)
                nc.vector.tensor_scalar(out=t2, in0=t2, scalar1=-0.5,
                                        scalar2=1.5, op0=OP.mult, op1=OP.add)
                nc.vector.tensor_mul(out=y1, in0=y1, in1=t2)
                if debug_taps and hp == 0:
                    nc.sync.dma_start(out=taps["scales"][:, :], in_=y1)
                nc.sync.dma_start(
                    out=scales_t[:].rearrange("a t -> (a t)")
                    .rearrange("(p f) -> p f", p=128),
                    in_=y1)
                # broadcast scales and normalize qtb/ktb in place (gpsimd)
                brcq = scp.tile([128, T], F32, tag="brcq", bufs=1)
                brck = scp.tile([128, T], F32, tag="brck", bufs=1)
                for h2 in range(2):
                    nc.sync.dma_start(
                        out=brcq[h2 * 64:(h2 + 1) * 64],
                        in_=bcast_rows(scales_t[:], h2 * T, 64, T))
                    nc.sync.dma_start(
                        out=brck[h2 * 64:(h2 + 1) * 64],
                        in_=bcast_rows(scales_t[:], (2 + h2) * T, 64, T))
                nc.gpsimd.tensor_mul(out=qtb[:, :], in0=qtb[:, :], in1=brcq)
                nc.gpsimd.tensor_mul(out=ktb[:, :], in0=ktb[:, :], in1=brck)

                if debug_taps and hp == 0:
                    nc.sync.dma_start(out=taps["qt"][:, :], in_=qtb[:, :])
                    nc.sync.dma_start(out=taps["kt"][:, :], in_=ktb[:, :])
                    nc.sync.dma_start(out=taps["vsb"][:, :, :], in_=vsb[:, :, :])

                # ---- phase 2: attention ----
                for qc in range(NQC):
                    qq = slice(qc * 512, (qc + 1) * 512)
                    ot0 = psp.tile([65, 512], F32, tag="ot0", bufs=1)
                    ot1 = psp.tile([65, 512], F32, tag="ot1", bufs=1)
                    for kt in range(NKT):
                        kk = slice(kt * 128, (kt + 1) * 128)
                        sps = psp.tile([128, 2, 512], F32, tag="sgrp", bufs=2)
                        nc.tensor.matmul(sps[:, 0], ktb[0:64, kk],
                                         qtb[0:64, qq], start=True, stop=True,
                                         skip_group_check=True)
                        nc.tensor.matmul(sps[:, 1], ktb[64:128, kk],
                                         qtb[64:128, qq], start=True,
                                         stop=True, skip_group_check=True)
                        stexp = atp.tile([128, 2, 512], BF, tag="stexp")
                        nc.scalar.activation(stexp, sps, AF.Exp, scale=0.125)
                        if debug_taps and hp == 0 and qc == 0 and kt == 0:
                            nc.sync.dma_start(out=taps["st"][:, :, :],
                                              in_=stexp[:, :, :])
                        nc.tensor.matmul(ot0, vsb[:, kt, 0:65], stexp[:, 0],
                                         start=(kt == 0), stop=(kt == NKT - 1),
                                         skip_group_check=True)
                        nc.tensor.matmul(ot1, vsb[:, kt, 65:130], stexp[:, 1],
                                         start=(kt == 0), stop=(kt == NKT - 1),
                                         skip_group_check=True)
                    if debug_taps and hp == 0 and qc == 0:
                        otc = scp.tile([65, 512], F32, tag="otc", bufs=1)
                        nc.vector.tensor_copy(out=otc, in_=ot0[:, :])
                        nc.sync.dma_start(out=taps["ot"][:, :], in_=otc)
                    # move O rows into ytb (h0 aligned; h1 via DMA shift)
                    nc.vector.tensor_copy(out=ytb_all[0:64, hp, qq],
                                          in_=ot0[0:64])
                    stg = scp.tile([64, 512], BF, tag="stg")
                    nc.vector.tensor_copy(out=stg, in_=ot1[0:64])
                    nc.sync.dma_start(out=ytb_all[64:128, hp, qq], in_=stg)
                    # sumexp rows -> se_c -> DRAM (packed reciprocal later)
                    se_c = scp.tile([65, 2, 512], F32, tag="sec")
                    nc.vector.tensor_copy(out=se_c[64:65, 0], in_=ot0[64:65])
                    nc.vector.tensor_copy(out=se_c[64:65, 1], in_=ot1[64:65])
                    nc.sync.dma_start(out=se_t[0:1, qq], in_=se_c[64:65, 0])
                    nc.sync.dma_start(out=se_t[1:2, qq], in_=se_c[64:65, 1])

                # packed reciprocal of sumexp, broadcast, normalize ytb
                FP2 = 2 * T // 128  # 32
                pkse = scp.tile([128, 3, FP2], F32, tag="pkse")
                nc.sync.dma_start(
                    out=pkse[:, 0],
                    in_=se_t[:].rearrange("a t -> (a t)")
                    .rearrange("(p f) -> p f", p=128))
                nc.vector.reciprocal_approx_accurate(
                    out=pkse[:, 2], in_=pkse[:, 0], scratch=pkse[:, 1])
                nc.sync.dma_start(
                    out=rcp_t[:].rearrange("a t -> (a t)")
                    .rearrange("(p f) -> p f", p=128),
                    in_=pkse[:, 2])
                brcy = scp.tile([128, T], F32, tag="brcy", bufs=1)
                for h2 in range(2):
                    nc.sync.dma_start(
                        out=brcy[h2 * 64:(h2 + 1) * 64],
                        in_=bcast_rows(rcp_t[:], h2 * T, 64, T))
                nc.gpsimd.tensor_mul(out=ytb_all[:, hp, :],
                                     in0=ytb_all[:, hp, :], in1=brcy)

            if debug_taps:
                nc.sync.dma_start(out=taps["yt"][:, :, :], in_=ytb_all[:, :, :])

            # ---- phase 3: Wo projection, partial output ----
            for oc in range(C // 512):
                for tt in range(T // 128):
                    pso = psp.tile([128, 512], F32, tag="mm", bufs=2)
                    for hp in range(NHP):
                        nc.tensor.matmul(pso,
                                         ytb_all[:, hp, tt * 128:(tt + 1) * 128],
                                         wo_sb[:, hp, oc * 512:(oc + 1) * 512],
                                         start=(hp == 0), stop=(hp == NHP - 1),
                                         skip_group_check=True)
                    ob = scp.tile([128, 512], F32, tag="ob")
                    nc.vector.tensor_copy(out=ob, in_=pso)
                    nc.sync.dma_start(
                        out=out_d[tt * 128:(tt + 1) * 128,
                                  oc * 512:(oc + 1) * 512],
                        in_=ob)

    nc.compile()
    return nc


def make_core_inputs(x, cos, sin, Wq, Wk, Wv, Wo, B, T):
    """Host-side sharding. Returns list of 8 input dicts.

    Core c handles batch c//2, head-group c%2 (8 heads each).
    """
    x = np.asarray(x, np.float32)
    cosT = np.asarray(cos, np.float32).reshape(T, D).T      # [64, T]
    sinT = np.asarray(sin, np.float32).reshape(T, D).T
    # sin_signed (dest-row order) = [-sin[0:32], sin[32:64]]; the kernel
    # indexes sin by SOURCE row (rot-half swaps 32-blocks), so permute:
    # sin_perm[s] = sin_signed[d(s)] with d = 32-block swap.
    sin_perm = np.concatenate([sinT[32:64], -sinT[0:32]], axis=0)
    cos2 = np.ascontiguousarray(
        np.concatenate([cosT, cosT], axis=0)).astype(BF16)
    sin2 = np.ascontiguousarray(
        np.concatenate([sin_perm, sin_perm], axis=0)).astype(BF16)
    ones2c = np.zeros((128, 2), BF16)
    ones2c[0:64, 0] = 1.0
    ones2c[64:128, 1] = 1.0
    ident = np.eye(128, dtype=BF16)
    onesk = np.ones((128, T // 128), BF16)

    Wq = np.asarray(Wq, np.float32)
    Wk = np.asarray(Wk, np.float32)
    Wv = np.asarray(Wv, np.float32)
    Wo = np.asarray(Wo, np.float32)

    xT_b = [np.ascontiguousarray(x[b].T).astype(BF16) for b in range(B)]
    wq_g = [np.ascontiguousarray(Wq[g * MG:(g + 1) * MG].T).astype(BF16)
            for g in range(2)]
    wk_g = [np.ascontiguousarray(Wk[g * MG:(g + 1) * MG].T).astype(BF16)
            for g in range(2)]
    wv_g = [np.ascontiguousarray(Wv[g * MG:(g + 1) * MG].T).astype(BF16)
            for g in range(2)]
    wo_g = [np.ascontiguousarray(Wo[:, g * MG:(g + 1) * MG].T).astype(BF16)
            for g in range(2)]

    in_maps = []
    for core in range(N_CORES):
        b, g = core // 2, core % 2
        in_maps.append({
            "xT": xT_b[b],
            "wq": wq_g[g],
            "wk": wk_g[g],
            "wv": wv_g[g],
            "wo": wo_g[g],
            "cos2": cos2,
            "sin2s": sin2,
            "ident": ident,
            "ones2c": ones2c,
            "onesk": onesk,
        })
    return in_maps


def kernel(x, cos, sin, Wq, Wk, Wv, Wo):
    from concourse.bass_utils import run_bass_kernel_spmd

    B, T = x.shape[0], x.shape[1]
    key = T
    if key not in _NC_CACHE:
        _NC_CACHE[key] = build_nc(T)
    nc = _NC_CACHE[key]
    in_maps = make_core_inputs(x, cos, sin, Wq, Wk, Wv, Wo, B, T)
    res = run_bass_kernel_spmd(nc, in_maps, core_ids=list(range(N_CORES)))
    out = np.zeros((B, T, C), np.float32)
    for core, r in enumerate(res.results):
        out[core // 2] += r["out"]
    return out


# revision 16
# speedup vs baseline: 2.0226x; 1.0209x over previous
"""Trainium2 Bass kernel for nn_MultiHeadAttention (B=4,T=2048,C=1024,H=16,D=64).

Sharding: batch x head-group. 8 cores = 4 batches x 2 groups of 8 heads.
Per core: one batch's x (loaded once, bf16), QKV column slices for its 8
heads (4 head-pairs), full attention, Wo row slice -> partial [T, C] output;
host sums the 2 partials per batch.

Per head-pair hp (2 heads on 128 partitions):
  qtb/ktb [128, T] bf16 : rows = 2 heads x 64 dims, cols = seq.
  vsb    [128, NKT, 130] bf16 : per key-tile [V_h0 | ones | V_h1 | ones].
  scores: two row-tiled K=64 matmuls (h0 rows 0-63, h1 rows 64-127) ->
          psum [128, 2, 512]; ACT exp (scale 1/8) -> stexp bf16.
  PV: lhsT = V_aug [128, 65] -> psum [65, 512]: rows 0:64 = O.T, row 64 =
      sumexp. 1/sumexp via packed reciprocal + DRAM broadcast.
  RMS norm scales via DVE fast-inverse-sqrt (bit trick + 2 Newton steps).
Wo phase at the end accumulates all 4 head-pairs per output tile.
"""
import sys

sys.path.insert(0, "/opt/trn_rl_repo")
import numpy as np
import ml_dtypes

N_CORES = 8
B_FULL, T_FULL, C = 4, 2048, 1024
H, D = 16, 64
HPC = H // 2                 # heads per core = 8
NHP = HPC // 2               # head-pairs per core = 4
M2 = 2 * D                   # 128 dims per head-pair
MG = HPC * D                 # 512 dims per core (head-group)
EPS = 1e-6
MAGIC = 0x5F3759DF

_NC_CACHE: dict = {}

BF16 = ml_dtypes.bfloat16


def build_nc(T: int, debug_taps: bool = False):
    import concourse.bass as bass
    import concourse.mybir as mybir
    from concourse import bacc
    from concourse.tile import TileContext

    F32 = mybir.dt.float32
    BF = mybir.dt.bfloat16
    I32 = mybir.dt.int32
    AF = mybir.ActivationFunctionType
    OP = mybir.AluOpType

    NCIN = C // 128              # 8 contraction tiles for projections
    CPB = T // 512               # 4 x 512-col chunks of T
    NKT = T // 128               # 16 key tiles
    NQC = T // 512               # 4 q chunks

    nc = bacc.Bacc("TRN2", target_bir_lowering=False, debug=False,
                   num_devices=N_CORES)

    xT_d = nc.dram_tensor("xT", [C, T], BF, kind="ExternalInput")
    wq_d = nc.dram_tensor("wq", [C, MG], BF, kind="ExternalInput")
    wk_d = nc.dram_tensor("wk", [C, MG], BF, kind="ExternalInput")
    wv_d = nc.dram_tensor("wv", [C, MG], BF, kind="ExternalInput")
    wo_d = nc.dram_tensor("wo", [MG, C], BF, kind="ExternalInput")
    cos_d = nc.dram_tensor("cos2", [128, T], BF, kind="ExternalInput")
    sin_d = nc.dram_tensor("sin2s", [128, T], BF, kind="ExternalInput")
    ident_d = nc.dram_tensor("ident", [128, 128], BF, kind="ExternalInput")
    ones2_d = nc.dram_tensor("ones2c", [128, 2], BF, kind="ExternalInput")
    onesk_d = nc.dram_tensor("onesk", [128, NKT], BF, kind="ExternalInput")
    out_d = nc.dram_tensor("out", [T, C], F32, kind="ExternalOutput")
    taps = {}
    if debug_taps:
        taps["qt"] = nc.dram_tensor("tap_qt", [128, T], BF, kind="ExternalOutput")
        taps["kt"] = nc.dram_tensor("tap_kt", [128, T], BF, kind="ExternalOutput")
        taps["vsb"] = nc.dram_tensor("tap_vsb", [128, NKT, 130], BF, kind="ExternalOutput")
        taps["scales"] = nc.dram_tensor("tap_scales", [128, 64], F32, kind="ExternalOutput")
        taps["st"] = nc.dram_tensor("tap_st", [128, 2, 512], BF, kind="ExternalOutput")
        taps["ot"] = nc.dram_tensor("tap_ot", [65, 512], F32, kind="ExternalOutput")
        taps["yt"] = nc.dram_tensor("tap_yt", [128, 4, T], BF, kind="ExternalOutput")

    def bcast_rows(tile_ap, row_elem_offset, n_rows, n_cols):
        # DRAM source AP broadcasting one row across n_rows partitions
        return bass.AP(tensor=tile_ap.tensor,
                       offset=tile_ap.offset + row_elem_offset,
                       ap=[[0, n_rows], [1, n_cols]])

    ROPE_SLICES = (((0, 32), (32, 64)), ((32, 64), (0, 32)),
                   ((64, 96), (96, 128)), ((96, 128), (64, 96)))

    with TileContext(nc) as tc:
        with (
            tc.tile_pool(name="const", bufs=1) as cp,
            tc.tile_pool(name="big", bufs=2) as bigp,
            tc.tile_pool(name="attn", bufs=3) as atp,
            tc.tile_pool(name="scr", bufs=2) as scp,
            tc.tile_pool(name="drs", bufs=2, space="DRAM") as drp,
            tc.tile_pool(name="ps", bufs=1, space="PSUM") as psp,
        ):
            x_sb = cp.tile([128, NCIN, T], BF, tag="x")
            wq_sb = cp.tile([128, NCIN, NHP, 128], BF, tag="wq")
            wk_sb = cp.tile([128, NCIN, NHP, 128], BF, tag="wk")
            wv_sb = cp.tile([128, NCIN, NHP, 128], BF, tag="wv")
            wo_sb = cp.tile([128, NHP, C], BF, tag="wo")
            cos_sb = cp.tile([128, T], BF, tag="cos")
            sin_sb = cp.tile([128, T], BF, tag="sin")
            ident = cp.tile([128, 128], BF, tag="ident")
            ones2 = cp.tile([128, 2], BF, tag="ones2")
            ytb_all = cp.tile([128, NHP, T], BF, tag="ytb")

            for cc4 in range(CPB):
                sl = slice(cc4 * 512, (cc4 + 1) * 512)
                nc.sync.dma_start(
                    out=x_sb[:, :, sl],
                    in_=xT_d[:, sl].rearrange("(co ci) t -> ci co t", ci=128))
            for w_sb, w_d in ((wq_sb, wq_d), (wk_sb, wk_d), (wv_sb, wv_d)):
                nc.sync.dma_start(
                    out=w_sb,
                    in_=w_d.rearrange("(co ci) (hp m) -> ci co hp m",
                                      ci=128, m=128))
            nc.sync.dma_start(
                out=wo_sb, in_=wo_d.rearrange("(hp ci) o -> ci hp o", ci=128))
            nc.sync.dma_start(out=cos_sb, in_=cos_d[:, :])
            nc.sync.dma_start(out=sin_sb, in_=sin_d[:, :])
            nc.sync.dma_start(out=ident, in_=ident_d[:, :])
            nc.sync.dma_start(out=ones2, in_=ones2_d[:, :])

            FP = 4 * T // 128   # 64 packed free dim (norm sums)
            FP2 = 2 * T // 128  # 32 packed free dim (sumexp)

            def emit_setup(hp):
                qtb = bigp.tile([128, T], BF, tag="qtb")
                ktb = bigp.tile([128, T], BF, tag="ktb")
                vsb = bigp.tile([128, NKT, 130], BF, tag="vsb")
                ss_q = bigp.tile([2, T], F32, tag="ssq", bufs=1)
                ss_k = bigp.tile([2, T], F32, tag="ssk", bufs=1)
                sums_t = drp.tile([4, T], F32, tag="sums")
                scales_t = drp.tile([4, T], F32, tag="scales")
                se_t = drp.tile([2, T], F32, tag="se")
                rcp_t = drp.tile([2, T], F32, tag="rcp")
                st = {"qtb": qtb, "ktb": ktb, "vsb": vsb, "ss_q": ss_q,
                      "ss_k": ss_k, "sums_t": sums_t, "scales_t": scales_t,
                      "se_t": se_t, "rcp_t": rcp_t}
                nc.sync.dma_start(out=vsb[:, :, 64:65], in_=onesk_d[:, :, None])
                nc.sync.dma_start(out=vsb[:, :, 129:130],
                                  in_=onesk_d[:, :, None])
                return st

            def emit_phase1_chunk(hp, c, st):
                # projections + RoPE + sumsq + V transpose for one 512-chunk
                cc = slice(c * 512, (c + 1) * 512)
                for w_sb, dst, ss_sb in ((wq_sb, st["qtb"], st["ss_q"]),
                                         (wk_sb, st["ktb"], st["ss_k"])):
                    ps = psp.tile([128, 512], F32, tag="mm", bufs=2)
                    for ci in range(NCIN):
                        nc.tensor.matmul(ps, w_sb[:, ci, hp], x_sb[:, ci, cc],
                                         start=(ci == 0),
                                         stop=(ci == NCIN - 1),
                                         skip_group_check=True)
                    pcp = scp.tile([128, 512], BF, tag="pcp")
                    nc.scalar.copy(out=pcp, in_=ps)
                    # RoPE: dst = pcp*cos + rot_half(pcp)*sin_signed.
                    # sin2s is host-permuted to SOURCE-row order so both
                    # SBUF inputs share a base partition; only the output
                    # AP carries the rotate-half partition shift.
                    rot = scp.tile([128, 512], BF, tag="rot")
                    for (d0, d1), (s0, s1) in ROPE_SLICES:
                        nc.vector.tensor_mul(
                            out=rot[d0:d1], in0=pcp[s0:s1],
                            in1=sin_sb[s0:s1, cc])
                    nc.vector.tensor_mul(out=dst[:, cc], in0=pcp,
                                         in1=cos_sb[:, cc])
                    nc.vector.tensor_add(out=dst[:, cc], in0=dst[:, cc],
                                         in1=rot)
                    # sum of squares over d (per head) via ones-matmul
                    sq = scp.tile([128, 512], BF, tag="sq")
                    nc.vector.tensor_mul(out=sq, in0=dst[:, cc],
                                         in1=dst[:, cc])
                    ps_ss = psp.tile([2, 512], F32, tag="mm", bufs=2)
                    nc.tensor.matmul(ps_ss, ones2, sq, start=True,
                                     stop=True, skip_group_check=True)
                    nc.vector.tensor_copy(out=ss_sb[:, cc], in_=ps_ss)

                # V: project then transpose to [t, d] layout
                vsb = st["vsb"]
                ps = psp.tile([128, 512], F32, tag="mm", bufs=2)
                for ci in range(NCIN):
                    nc.tensor.matmul(ps, wv_sb[:, ci, hp], x_sb[:, ci, cc],
                                     start=(ci == 0), stop=(ci == NCIN - 1),
                                     skip_group_check=True)
                vtmp = scp.tile([128, 512], BF, tag="vtmp")
                nc.scalar.copy(out=vtmp, in_=ps)
                for i in range(4):
                    kt_idx = c * 4 + i
                    ps_t = psp.tile([128, 128], BF, tag="mm", bufs=2)
                    nc.tensor.transpose(ps_t, vtmp[:, i * 128:(i + 1) * 128],
                                        ident)
                    dst_ap = bass.AP(
                        tensor=vsb.tensor,
                        offset=vsb[:, kt_idx, 0:64].offset,
                        ap=[vsb[:, kt_idx, 0:64].ap[0], [65, 2], [1, 64]])
                    nc.vector.tensor_copy(
                        out=dst_ap,
                        in_=ps_t[:, :].rearrange("p (a b) -> p a b", a=2))

            def emit_scales(hp, st):
                # rsqrt of mean-square via fast-inverse-sqrt (DVE only),
                # then broadcast and normalize qtb/ktb in place
                sums_t, scales_t = st["sums_t"], st["scales_t"]
                nc.sync.dma_start(out=sums_t[0:2, :], in_=st["ss_q"])
                nc.sync.dma_start(out=sums_t[2:4, :], in_=st["ss_k"])
                pk = scp.tile([128, 8, FP], F32, tag="pk")
                ms, t1, y1, t2 = (pk[:, j] for j in range(1, 5))
                hi = pk[:, 5].bitcast(I32)
                y0i = pk[:, 6].bitcast(I32)
                nc.sync.dma_start(
                    out=pk[:, 0],
                    in_=sums_t[:].rearrange("a t -> (a t)")
                    .rearrange("(p f) -> p f", p=128))
                nc.vector.tensor_scalar(out=ms, in0=pk[:, 0], scalar1=1.0 / D,
                                        scalar2=EPS, op0=OP.mult, op1=OP.add)
                # y0 = bitcast(MAGIC - (bits(ms) >> 1)); 2 Newton steps
                nc.vector.tensor_scalar(out=hi, in0=ms.bitcast(I32),
                                        scalar1=1, scalar2=None,
                                        op0=OP.logical_shift_right)
                nc.vector.tensor_scalar(out=y0i, in0=hi, scalar1=-1.0,
                                        scalar2=float(MAGIC),
                                        op0=OP.mult, op1=OP.add)
                y0 = pk[:, 6]
                nc.vector.tensor_mul(out=t1, in0=ms, in1=y0)
                nc.vector.tensor_mul(out=t1, in0=t1, in1=y0)
                nc.vector.tensor_scalar(out=t1, in0=t1, scalar1=-0.5,
                                        scalar2=1.5, op0=OP.mult, op1=OP.add)
                nc.vector.tensor_mul(out=y1, in0=y0, in1=t1)
                nc.vector.tensor_mul(out=t2, in0=ms, in1=y1)
                nc.vector.tensor_mul(out=t2, in0=t2, in1=y1)
                nc.vector.tensor_scalar(out=t2, in0=t2, scalar1=-0.5,
                                        scalar2=1.5, op0=OP.mult, op1=OP.add)
                nc.vector.tensor_mul(out=y1, in0=y1, in1=t2)
                if debug_taps and hp == 0:
                    nc.sync.dma_start(out=taps["scales"][:, :], in_=y1)
                nc.sync.dma_start(
                    out=scales_t[:].rearrange("a t -> (a t)")
                    .rearrange("(p f) -> p f", p=128),
                    in_=y1)
                brcq = scp.tile([128, T], F32, tag="brcq", bufs=1)
                brck = scp.tile([128, T], F32, tag="brck", bufs=1)
                for h2 in range(2):
                    nc.sync.dma_start(
                        out=brcq[h2 * 64:(h2 + 1) * 64],
                        in_=bcast_rows(scales_t[:], h2 * T, 64, T))
                    nc.sync.dma_start(
                        out=brck[h2 * 64:(h2 + 1) * 64],
                        in_=bcast_rows(scales_t[:], (2 + h2) * T, 64, T))
                nc.gpsimd.tensor_mul(out=st["qtb"][:, :], in0=st["qtb"][:, :],
                                     in1=brcq)
                nc.gpsimd.tensor_mul(out=st["ktb"][:, :], in0=st["ktb"][:, :],
                                     in1=brck)
                if debug_taps and hp == 0:
                    nc.sync.dma_start(out=taps["qt"][:, :], in_=st["qtb"][:, :])
                    nc.sync.dma_start(out=taps["kt"][:, :], in_=st["ktb"][:, :])
                    nc.sync.dma_start(out=taps["vsb"][:, :, :],
                                      in_=st["vsb"][:, :, :])

            def emit_attn_qc(hp, qc, st):
                qtb, ktb, vsb, se_t = (st["qtb"], st["ktb"], st["vsb"],
                                       st["se_t"])
                qq = slice(qc * 512, (qc + 1) * 512)
                ot0 = psp.tile([65, 512], F32, tag="ot0", bufs=1)
                ot1 = psp.tile([65, 512], F32, tag="ot1", bufs=1)
                for kt in range(NKT):
                    kk = slice(kt * 128, (kt + 1) * 128)
                    sps = psp.tile([128, 2, 512], F32, tag="sgrp", bufs=2)
                    nc.tensor.matmul(sps[:, 0], ktb[0:64, kk],
                                     qtb[0:64, qq], start=True, stop=True,
                                     skip_group_check=True)
                    nc.tensor.matmul(sps[:, 1], ktb[64:128, kk],
                                     qtb[64:128, qq], start=True,
                                     stop=True, skip_group_check=True)
                    stexp = atp.tile([128, 2, 512], BF, tag="stexp")
                    nc.scalar.activation(stexp, sps, AF.Exp, scale=0.125)
                    if debug_taps and hp == 0 and qc == 0 and kt == 0:
                        nc.sync.dma_start(out=taps["st"][:, :, :],
                                          in_=stexp[:, :, :])
                    nc.tensor.matmul(ot0, vsb[:, kt, 0:65], stexp[:, 0],
                                     start=(kt == 0), stop=(kt == NKT - 1),
                                     skip_group_check=True)
                    nc.tensor.matmul(ot1, vsb[:, kt, 65:130], stexp[:, 1],
                                     start=(kt == 0), stop=(kt == NKT - 1),
                                     skip_group_check=True)
                if debug_taps and hp == 0 and qc == 0:
                    otc = scp.tile([65, 512], F32, tag="otc", bufs=1)
                    nc.vector.tensor_copy(out=otc, in_=ot0[:, :])
                    nc.sync.dma_start(out=taps["ot"][:, :], in_=otc)
                # move O rows into ytb (h0 aligned; h1 via DMA shift)
                nc.vector.tensor_copy(out=ytb_all[0:64, hp, qq],
                                      in_=ot0[0:64])
                stg = scp.tile([64, 512], BF, tag="stg")
                nc.vector.tensor_copy(out=stg, in_=ot1[0:64])
                nc.sync.dma_start(out=ytb_all[64:128, hp, qq], in_=stg)
                # sumexp rows -> se_c -> DRAM (packed reciprocal later)
                se_c = scp.tile([65, 2, 512], F32, tag="sec")
                nc.vector.tensor_copy(out=se_c[64:65, 0], in_=ot0[64:65])
                nc.vector.tensor_copy(out=se_c[64:65, 1], in_=ot1[64:65])
                nc.sync.dma_start(out=se_t[0:1, qq], in_=se_c[64:65, 0])
                nc.sync.dma_start(out=se_t[1:2, qq], in_=se_c[64:65, 1])

            def emit_finish(hp, st):
                # packed reciprocal of sumexp, broadcast, normalize ytb
                se_t, rcp_t = st["se_t"], st["rcp_t"]
                pkse = scp.tile([128, 3, FP2], F32, tag="pkse")
                nc.sync.dma_start(
                    out=pkse[:, 0],
                    in_=se_t[:].rearrange("a t -> (a t)")
                    .rearrange("(p f) -> p f", p=128))
                nc.vector.reciprocal_approx_accurate(
                    out=pkse[:, 2], in_=pkse[:, 0], scratch=pkse[:, 1])
                nc.sync.dma_start(
                    out=rcp_t[:].rearrange("a t -> (a t)")
                    .rearrange("(p f) -> p f", p=128),
                    in_=pkse[:, 2])
                brcy = scp.tile([128, T], F32, tag="brcy", bufs=1)
                for h2 in range(2):
                    nc.sync.dma_start(
                        out=brcy[h2 * 64:(h2 + 1) * 64],
                        in_=bcast_rows(rcp_t[:], h2 * T, 64, T))
                nc.gpsimd.tensor_mul(out=ytb_all[:, hp, :],
                                     in0=ytb_all[:, hp, :], in1=brcy)

            # software pipeline: attention(hp) overlaps phase 1 of hp+1
            states = {0: emit_setup(0)}
            for c in range(CPB):
                emit_phase1_chunk(0, c, states[0])
            emit_scales(0, states[0])
            for hp in range(NHP):
                nxt = hp + 1
                if nxt < NHP:
                    states[nxt] = emit_setup(nxt)
                for qc in range(NQC):
                    emit_attn_qc(hp, qc, states[hp])
                    if nxt < NHP:
                        emit_phase1_chunk(nxt, qc, states[nxt])
                if nxt < NHP:
                    emit_scales(nxt, states[nxt])
                emit_finish(hp, states[hp])
                del states[hp]

            if debug_taps:
                nc.sync.dma_start(out=taps["yt"][:, :, :], in_=ytb_all[:, :, :])

            # ---- phase 3: Wo projection, partial output ----
            for oc in range(C // 512):
                for tt in range(T // 128):
                    pso = psp.tile([128, 512], F32, tag="mm", bufs=2)
                    for hp in range(NHP):
                        nc.tensor.matmul(pso,
                                         ytb_all[:, hp, tt * 128:(tt + 1) * 128],
                                         wo_sb[:, hp, oc * 512:(oc + 1) * 512],
                                         start=(hp == 0), stop=(hp == NHP - 1),
                                         skip_group_check=True)
                    ob = scp.tile([128, 512], F32, tag="ob")
                    nc.vector.tensor_copy(out=ob, in_=pso)
                    nc.sync.dma_start(
                        out=out_d[tt * 128:(tt + 1) * 128,
                                  oc * 512:(oc + 1) * 512],
                        in_=ob)

    nc.compile()
    return nc


def make_core_inputs(x, cos, sin, Wq, Wk, Wv, Wo, B, T):
    """Host-side sharding. Returns list of 8 input dicts.

    Core c handles batch c//2, head-group c%2 (8 heads each).
    """
    x = np.asarray(x, np.float32)
    cosT = np.asarray(cos, np.float32).reshape(T, D).T      # [64, T]
    sinT = np.asarray(sin, np.float32).reshape(T, D).T
    # sin_signed (dest-row order) = [-sin[0:32], sin[32:64]]; the kernel
    # indexes sin by SOURCE row (rot-half swaps 32-blocks), so permute:
    # sin_perm[s] = sin_signed[d(s)] with d = 32-block swap.
    sin_perm = np.concatenate([sinT[32:64], -sinT[0:32]], axis=0)
    cos2 = np.ascontiguousarray(
        np.concatenate([cosT, cosT], axis=0)).astype(BF16)
    sin2 = np.ascontiguousarray(
        np.concatenate([sin_perm, sin_perm], axis=0)).astype(BF16)
    ones2c = np.zeros((128, 2), BF16)
    ones2c[0:64, 0] = 1.0
    ones2c[64:128, 1] = 1.0
    ident = np.eye(128, dtype=BF16)
    onesk = np.ones((128, T // 128), BF16)

    Wq = np.asarray(Wq, np.float32)
    Wk = np.asarray(Wk, np.float32)
    Wv = np.asarray(Wv, np.float32)
    Wo = np.asarray(Wo, np.float32)

    xT_b = [np.ascontiguousarray(x[b].T).astype(BF16) for b in range(B)]
    wq_g = [np.ascontiguousarray(Wq[g * MG:(g + 1) * MG].T).astype(BF16)
            for g in range(2)]
    wk_g = [np.ascontiguousarray(Wk[g * MG:(g + 1) * MG].T).astype(BF16)
            for g in range(2)]
    wv_g = [np.ascontiguousarray(Wv[g * MG:(g + 1) * MG].T).astype(BF16)
            for g in range(2)]
    wo_g = [np.ascontiguousarray(Wo[:, g * MG:(g + 1) * MG].T).astype(BF16)
            for g in range(2)]

    in_maps = []
    for core in range(N_CORES):
        b, g = core // 2, core % 2
        in_maps.append({
            "xT": xT_b[b],
            "wq": wq_g[g],
            "wk": wk_g[g],
            "wv": wv_g[g],
            "wo": wo_g[g],
            "cos2": cos2,
            "sin2s": sin2,
            "ident": ident,
            "ones2c": ones2c,
            "onesk": onesk,
        })
    return in_maps


def kernel(x, cos, sin, Wq, Wk, Wv, Wo):
    from concourse.bass_utils import run_bass_kernel_spmd

    B, T = x.shape[0], x.shape[1]
    key = T
    if key not in _NC_CACHE:
        _NC_CACHE[key] = build_nc(T)
    nc = _NC_CACHE[key]
    in_maps = make_core_inputs(x, cos, sin, Wq, Wk, Wv, Wo, B, T)
    res = run_bass_kernel_spmd(nc, in_maps, core_ids=list(range(N_CORES)))
    out = np.zeros((B, T, C), np.float32)
    for core, r in enumerate(res.results):
        out[core // 2] += r["out"]
    return out


# revision 18
# speedup vs baseline: 2.2030x; 1.0892x over previous
"""Trainium2 Bass kernel for nn_MultiHeadAttention (B=4,T=2048,C=1024,H=16,D=64).

Sharding: batch x head-group. 8 cores = 4 batches x 2 groups of 8 heads.
Per core: one batch's x (loaded once, bf16), QKV column slices for its 8
heads (4 head-pairs), full attention, Wo row slice -> partial [T, C] output;
host sums the 2 partials per batch.

Per head-pair hp (2 heads on 128 partitions):
  qtb/ktb [128, T] bf16 : rows = 2 heads x 64 dims, cols = seq.
  vsb    [128, NKT, 130] bf16 : per key-tile [V_h0 | ones | V_h1 | ones].
  scores: two row-tiled K=64 matmuls (h0 rows 0-63, h1 rows 64-127) ->
          psum [128, 2, 512]; ACT exp (scale 1/8) -> stexp bf16.
  PV: lhsT = V_aug [128, 65] -> psum [65, 512]: rows 0:64 = O.T, row 64 =
      sumexp. 1/sumexp via packed reciprocal + DRAM broadcast.
  RMS norm scales via DVE fast-inverse-sqrt (bit trick + 2 Newton steps).
Wo phase at the end accumulates all 4 head-pairs per output tile.
"""
import sys

sys.path.insert(0, "/opt/trn_rl_repo")
import numpy as np
import ml_dtypes

N_CORES = 8
B_FULL, T_FULL, C = 4, 2048, 1024
H, D = 16, 64
HPC = H // 2                 # heads per core = 8
NHP = HPC // 2               # head-pairs per core = 4
M2 = 2 * D                   # 128 dims per head-pair
MG = HPC * D                 # 512 dims per core (head-group)
EPS = 1e-6
MAGIC = 0x5F3759DF

_NC_CACHE: dict = {}

BF16 = ml_dtypes.bfloat16


def build_nc(T: int, debug_taps: bool = False):
    import concourse.bass as bass
    import concourse.mybir as mybir
    from concourse import bacc
    from concourse.tile import TileContext

    F32 = mybir.dt.float32
    BF = mybir.dt.bfloat16
    I32 = mybir.dt.int32
    AF = mybir.ActivationFunctionType
    OP = mybir.AluOpType

    NCIN = C // 128              # 8 contraction tiles for projections
    CPB = T // 512               # 4 x 512-col chunks of T
    NKT = T // 128               # 16 key tiles
    NQC = T // 512               # 4 q chunks

    nc = bacc.Bacc("TRN2", target_bir_lowering=False, debug=False,
                   num_devices=N_CORES)

    xT_d = nc.dram_tensor("xT", [128, CPB, NCIN, 512], BF,
                          kind="ExternalInput")
    wq_d = nc.dram_tensor("wq", [128, NCIN, NHP, 128], BF,
                          kind="ExternalInput")
    wk_d = nc.dram_tensor("wk", [128, NCIN, NHP, 128], BF,
                          kind="ExternalInput")
    wv_d = nc.dram_tensor("wv", [128, NCIN, NHP, 128], BF,
                          kind="ExternalInput")
    wo_d = nc.dram_tensor("wo", [128, NHP, C], BF, kind="ExternalInput")
    cos_d = nc.dram_tensor("cos2", [128, T], BF, kind="ExternalInput")
    sin_d = nc.dram_tensor("sin2s", [128, T], BF, kind="ExternalInput")
    ident_d = nc.dram_tensor("ident", [128, 128], BF, kind="ExternalInput")
    ones2_d = nc.dram_tensor("ones2c", [128, 2], BF, kind="ExternalInput")
    onesk_d = nc.dram_tensor("onesk", [128, NKT], BF, kind="ExternalInput")
    out_d = nc.dram_tensor("out", [T, C], F32, kind="ExternalOutput")
    taps = {}
    if debug_taps:
        taps["qt"] = nc.dram_tensor("tap_qt", [128, T], BF, kind="ExternalOutput")
        taps["kt"] = nc.dram_tensor("tap_kt", [128, T], BF, kind="ExternalOutput")
        taps["vsb"] = nc.dram_tensor("tap_vsb", [128, NKT, 130], BF, kind="ExternalOutput")
        taps["scales"] = nc.dram_tensor("tap_scales", [128, 64], F32, kind="ExternalOutput")
        taps["st"] = nc.dram_tensor("tap_st", [128, 2, 512], BF, kind="ExternalOutput")
        taps["ot"] = nc.dram_tensor("tap_ot", [65, 512], F32, kind="ExternalOutput")
        taps["yt"] = nc.dram_tensor("tap_yt", [128, 4, T], BF, kind="ExternalOutput")

    def bcast_rows(tile_ap, row_elem_offset, n_rows, n_cols):
        # DRAM source AP broadcasting one row across n_rows partitions
        return bass.AP(tensor=tile_ap.tensor,
                       offset=tile_ap.offset + row_elem_offset,
                       ap=[[0, n_rows], [1, n_cols]])

    ROPE_SLICES = (((0, 32), (32, 64)), ((32, 64), (0, 32)),
                   ((64, 96), (96, 128)), ((96, 128), (64, 96)))

    with TileContext(nc) as tc:
        with (
            tc.tile_pool(name="const", bufs=1) as cp,
            tc.tile_pool(name="big", bufs=2) as bigp,
            tc.tile_pool(name="attn", bufs=3) as atp,
            tc.tile_pool(name="scr", bufs=2) as scp,
            tc.tile_pool(name="drs", bufs=2, space="DRAM") as drp,
            tc.tile_pool(name="ps", bufs=1, space="PSUM") as psp,
        ):
            x_sb = cp.tile([128, CPB, NCIN, 512], BF, tag="x")
            wq_sb = cp.tile([128, NCIN, NHP, 128], BF, tag="wq")
            wk_sb = cp.tile([128, NCIN, NHP, 128], BF, tag="wk")
            wv_sb = cp.tile([128, NCIN, NHP, 128], BF, tag="wv")
            wo_sb = cp.tile([128, NHP, C], BF, tag="wo")
            cos_sb = cp.tile([128, T], BF, tag="cos")
            sin_sb = cp.tile([128, T], BF, tag="sin")
            ident = cp.tile([128, 128], BF, tag="ident")
            ones2 = cp.tile([128, 2], BF, tag="ones2")
            ytb_all = cp.tile([128, NHP, T], BF, tag="ytb")

            for cc4 in range(CPB):
                nc.sync.dma_start(out=x_sb[:, cc4], in_=xT_d[:, cc4])
            for w_sb, w_d in ((wq_sb, wq_d), (wk_sb, wk_d), (wv_sb, wv_d)):
                nc.sync.dma_start(out=w_sb, in_=w_d[:, :, :, :])
            nc.sync.dma_start(out=wo_sb, in_=wo_d[:, :, :])
            nc.sync.dma_start(out=cos_sb, in_=cos_d[:, :])
            nc.sync.dma_start(out=sin_sb, in_=sin_d[:, :])
            nc.sync.dma_start(out=ident, in_=ident_d[:, :])
            nc.sync.dma_start(out=ones2, in_=ones2_d[:, :])

            FP = 4 * T // 128   # 64 packed free dim (norm sums)
            FP2 = 2 * T // 128  # 32 packed free dim (sumexp)

            def emit_setup(hp):
                qtb = bigp.tile([128, T], BF, tag="qtb")
                ktb = bigp.tile([128, T], BF, tag="ktb")
                vsb = bigp.tile([128, NKT, 130], BF, tag="vsb")
                ss_q = bigp.tile([2, T], F32, tag="ssq", bufs=1)
                ss_k = bigp.tile([2, T], F32, tag="ssk", bufs=1)
                sums_t = drp.tile([4, T], F32, tag="sums")
                scales_t = drp.tile([4, T], F32, tag="scales")
                se_t = drp.tile([2, T], F32, tag="se")
                rcp_t = drp.tile([2, T], F32, tag="rcp")
                st = {"qtb": qtb, "ktb": ktb, "vsb": vsb, "ss_q": ss_q,
                      "ss_k": ss_k, "sums_t": sums_t, "scales_t": scales_t,
                      "se_t": se_t, "rcp_t": rcp_t}
                nc.sync.dma_start(out=vsb[:, :, 64:65], in_=onesk_d[:, :, None])
                nc.sync.dma_start(out=vsb[:, :, 129:130],
                                  in_=onesk_d[:, :, None])
                return st

            def emit_phase1_chunk(hp, c, st):
                # projections + RoPE + sumsq + V transpose for one 512-chunk
                cc = slice(c * 512, (c + 1) * 512)
                for w_sb, dst, ss_sb in ((wq_sb, st["qtb"], st["ss_q"]),
                                         (wk_sb, st["ktb"], st["ss_k"])):
                    ps = psp.tile([128, 512], F32, tag="mm", bufs=2)
                    for ci in range(NCIN):
                        nc.tensor.matmul(ps, w_sb[:, ci, hp],
                                         x_sb[:, c, ci, :],
                                         start=(ci == 0),
                                         stop=(ci == NCIN - 1),
                                         skip_group_check=True)
                    pcp = scp.tile([128, 512], BF, tag="pcp")
                    nc.scalar.copy(out=pcp, in_=ps)
                    # RoPE: dst = pcp*cos + rot_half(pcp)*sin_signed.
                    # sin2s is host-permuted to SOURCE-row order so both
                    # SBUF inputs share a base partition; only the output
                    # AP carries the rotate-half partition shift.
                    rot = scp.tile([128, 512], BF, tag="rot")
                    for (d0, d1), (s0, s1) in ROPE_SLICES:
                        nc.vector.tensor_mul(
                            out=rot[d0:d1], in0=pcp[s0:s1],
                            in1=sin_sb[s0:s1, cc])
                    nc.vector.tensor_mul(out=dst[:, cc], in0=pcp,
                                         in1=cos_sb[:, cc])
                    nc.vector.tensor_add(out=dst[:, cc], in0=dst[:, cc],
                                         in1=rot)
                    # sum of squares over d (per head) via ones-matmul
                    sq = scp.tile([128, 512], BF, tag="sq")
                    nc.vector.tensor_mul(out=sq, in0=dst[:, cc],
                                         in1=dst[:, cc])
                    ps_ss = psp.tile([2, 512], F32, tag="mm", bufs=2)
                    nc.tensor.matmul(ps_ss, ones2, sq, start=True,
                                     stop=True, skip_group_check=True)
                    nc.vector.tensor_copy(out=ss_sb[:, cc], in_=ps_ss)

                # V: project then transpose to [t, d] layout
                vsb = st["vsb"]
                ps = psp.tile([128, 512], F32, tag="mm", bufs=2)
                for ci in range(NCIN):
                    nc.tensor.matmul(ps, wv_sb[:, ci, hp], x_sb[:, c, ci, :],
                                     start=(ci == 0), stop=(ci == NCIN - 1),
                                     skip_group_check=True)
                vtmp = scp.tile([128, 512], BF, tag="vtmp")
                nc.scalar.copy(out=vtmp, in_=ps)
                for i in range(4):
                    kt_idx = c * 4 + i
                    ps_t = psp.tile([128, 128], BF, tag="mm", bufs=2)
                    nc.tensor.transpose(ps_t, vtmp[:, i * 128:(i + 1) * 128],
                                        ident)
                    dst_ap = bass.AP(
                        tensor=vsb.tensor,
                        offset=vsb[:, kt_idx, 0:64].offset,
                        ap=[vsb[:, kt_idx, 0:64].ap[0], [65, 2], [1, 64]])
                    nc.vector.tensor_copy(
                        out=dst_ap,
                        in_=ps_t[:, :].rearrange("p (a b) -> p a b", a=2))

            def emit_scales(hp, st):
                # rsqrt of mean-square via fast-inverse-sqrt (DVE only),
                # then broadcast and normalize qtb/ktb in place
                sums_t, scales_t = st["sums_t"], st["scales_t"]
                nc.sync.dma_start(out=sums_t[0:2, :], in_=st["ss_q"])
                nc.sync.dma_start(out=sums_t[2:4, :], in_=st["ss_k"])
                pk = scp.tile([128, 8, FP], F32, tag="pk")
                ms, t1, y1, t2 = (pk[:, j] for j in range(1, 5))
                hi = pk[:, 5].bitcast(I32)
                y0i = pk[:, 6].bitcast(I32)
                nc.sync.dma_start(
                    out=pk[:, 0],
                    in_=sums_t[:].rearrange("a t -> (a t)")
                    .rearrange("(p f) -> p f", p=128))
                nc.vector.tensor_scalar(out=ms, in0=pk[:, 0], scalar1=1.0 / D,
                                        scalar2=EPS, op0=OP.mult, op1=OP.add)
                # y0 = bitcast(MAGIC - (bits(ms) >> 1)); 2 Newton steps
                nc.vector.tensor_scalar(out=hi, in0=ms.bitcast(I32),
                                        scalar1=1, scalar2=None,
                                        op0=OP.logical_shift_right)
                nc.vector.tensor_scalar(out=y0i, in0=hi, scalar1=-1.0,
                                        scalar2=float(MAGIC),
                                        op0=OP.mult, op1=OP.add)
                y0 = pk[:, 6]
                nc.vector.tensor_mul(out=t1, in0=ms, in1=y0)
                nc.vector.tensor_mul(out=t1, in0=t1, in1=y0)
                nc.vector.tensor_scalar(out=t1, in0=t1, scalar1=-0.5,
                                        scalar2=1.5, op0=OP.mult, op1=OP.add)
                nc.vector.tensor_mul(out=y1, in0=y0, in1=t1)
                nc.vector.tensor_mul(out=t2, in0=ms, in1=y1)
                nc.vector.tensor_mul(out=t2, in0=t2, in1=y1)
                nc.vector.tensor_scalar(out=t2, in0=t2, scalar1=-0.5,
                                        scalar2=1.5, op0=OP.mult, op1=OP.add)
                nc.vector.tensor_mul(out=y1, in0=y1, in1=t2)
                if debug_taps and hp == 0:
                    nc.sync.dma_start(out=taps["scales"][:, :], in_=y1)
                nc.sync.dma_start(
                    out=scales_t[:].rearrange("a t -> (a t)")
                    .rearrange("(p f) -> p f", p=128),
                    in_=y1)
                brcq = scp.tile([128, T], F32, tag="brcq", bufs=1)
                brck = scp.tile([128, T], F32, tag="brck", bufs=1)
                for h2 in range(2):
                    nc.sync.dma_start(
                        out=brcq[h2 * 64:(h2 + 1) * 64],
                        in_=bcast_rows(scales_t[:], h2 * T, 64, T))
                    nc.sync.dma_start(
                        out=brck[h2 * 64:(h2 + 1) * 64],
                        in_=bcast_rows(scales_t[:], (2 + h2) * T, 64, T))
                nc.vector.tensor_mul(out=st["qtb"][:, :], in0=st["qtb"][:, :],
                                     in1=brcq)
                nc.vector.tensor_mul(out=st["ktb"][:, :], in0=st["ktb"][:, :],
                                     in1=brck)
                if debug_taps and hp == 0:
                    nc.sync.dma_start(out=taps["qt"][:, :], in_=st["qtb"][:, :])
                    nc.sync.dma_start(out=taps["kt"][:, :], in_=st["ktb"][:, :])
                    nc.sync.dma_start(out=taps["vsb"][:, :, :],
                                      in_=st["vsb"][:, :, :])

            def emit_attn_qc(hp, qc, st):
                qtb, ktb, vsb, se_t = (st["qtb"], st["ktb"], st["vsb"],
                                       st["se_t"])
                qq = slice(qc * 512, (qc + 1) * 512)
                ot0 = psp.tile([65, 512], F32, tag="ot0", bufs=1)
                ot1 = psp.tile([65, 512], F32, tag="ot1", bufs=1)
                for kt in range(NKT):
                    kk = slice(kt * 128, (kt + 1) * 128)
                    sps = psp.tile([128, 2, 512], F32, tag="sgrp", bufs=2)
                    nc.tensor.matmul(sps[:, 0], ktb[0:64, kk],
                                     qtb[0:64, qq], start=True, stop=True,
                                     skip_group_check=True)
                    nc.tensor.matmul(sps[:, 1], ktb[64:128, kk],
                                     qtb[64:128, qq], start=True,
                                     stop=True, skip_group_check=True)
                    stexp = atp.tile([128, 2, 512], BF, tag="stexp")
                    nc.scalar.activation(stexp, sps, AF.Exp, scale=0.125)
                    if debug_taps and hp == 0 and qc == 0 and kt == 0:
                        nc.sync.dma_start(out=taps["st"][:, :, :],
                                          in_=stexp[:, :, :])
                    nc.tensor.matmul(ot0, vsb[:, kt, 0:65], stexp[:, 0],
                                     start=(kt == 0), stop=(kt == NKT - 1),
                                     skip_group_check=True)
                    nc.tensor.matmul(ot1, vsb[:, kt, 65:130], stexp[:, 1],
                                     start=(kt == 0), stop=(kt == NKT - 1),
                                     skip_group_check=True)
                if debug_taps and hp == 0 and qc == 0:
                    otc = scp.tile([65, 512], F32, tag="otc", bufs=1)
                    nc.vector.tensor_copy(out=otc, in_=ot0[:, :])
                    nc.sync.dma_start(out=taps["ot"][:, :], in_=otc)
                # move O rows into ytb (h0 aligned; h1 via DMA shift)
                nc.vector.tensor_copy(out=ytb_all[0:64, hp, qq],
                                      in_=ot0[0:64])
                stg = scp.tile([64, 512], BF, tag="stg")
                nc.vector.tensor_copy(out=stg, in_=ot1[0:64])
                nc.sync.dma_start(out=ytb_all[64:128, hp, qq], in_=stg)
                # sumexp rows -> se_c -> DRAM (packed reciprocal later)
                se_c = scp.tile([65, 2, 512], F32, tag="sec")
                nc.vector.tensor_copy(out=se_c[64:65, 0], in_=ot0[64:65])
                nc.vector.tensor_copy(out=se_c[64:65, 1], in_=ot1[64:65])
                nc.sync.dma_start(out=se_t[0:1, qq], in_=se_c[64:65, 0])
                nc.sync.dma_start(out=se_t[1:2, qq], in_=se_c[64:65, 1])

            def emit_finish(hp, st):
                # packed reciprocal of sumexp, broadcast, normalize ytb
                se_t, rcp_t = st["se_t"], st["rcp_t"]
                pkse = scp.tile([128, 3, FP2], F32, tag="pkse")
                nc.sync.dma_start(
                    out=pkse[:, 0],
                    in_=se_t[:].rearrange("a t -> (a t)")
                    .rearrange("(p f) -> p f", p=128))
                nc.vector.reciprocal_approx_accurate(
                    out=pkse[:, 2], in_=pkse[:, 0], scratch=pkse[:, 1])
                nc.sync.dma_start(
                    out=rcp_t[:].rearrange("a t -> (a t)")
                    .rearrange("(p f) -> p f", p=128),
                    in_=pkse[:, 2])
                brcy = scp.tile([128, T], F32, tag="brcy", bufs=1)
                for h2 in range(2):
                    nc.sync.dma_start(
                        out=brcy[h2 * 64:(h2 + 1) * 64],
                        in_=bcast_rows(rcp_t[:], h2 * T, 64, T))
                nc.gpsimd.tensor_mul(out=ytb_all[:, hp, :],
                                     in0=ytb_all[:, hp, :], in1=brcy)

            # software pipeline: attention(hp) overlaps phase 1 of hp+1.
            # phase-1 chunks are front-loaded (2 at qc=0) so the scales
            # chain for hp+1 completes before attention(hp) finishes.
            states = {0: emit_setup(0)}
            for c in range(CPB):
                emit_phase1_chunk(0, c, states[0])
            emit_scales(0, states[0])
            for hp in range(NHP):
                nxt = hp + 1
                if nxt < NHP:
                    states[nxt] = emit_setup(nxt)
                for qc in range(NQC):
                    emit_attn_qc(hp, qc, states[hp])
                    if nxt < NHP:
                        if qc == 0:
                            emit_phase1_chunk(nxt, 0, states[nxt])
                            emit_phase1_chunk(nxt, 1, states[nxt])
                        elif qc == 1:
                            emit_phase1_chunk(nxt, 2, states[nxt])
                        elif qc == 2:
                            emit_phase1_chunk(nxt, 3, states[nxt])
                            emit_scales(nxt, states[nxt])
                emit_finish(hp, states[hp])
                del states[hp]

            if debug_taps:
                nc.sync.dma_start(out=taps["yt"][:, :, :], in_=ytb_all[:, :, :])

            # ---- phase 3: Wo projection, partial output ----
            WO_TAGS = ("mm", "sgrp", "ot0", "ot1")
            for oc in range(C // 512):
                for tt in range(T // 128):
                    pso = psp.tile([128, 512], F32,
                                   tag=WO_TAGS[(oc * 16 + tt) % 4],
                                   bufs=(2 if (oc * 16 + tt) % 4 < 2 else 1))
                    for hp in range(NHP):
                        nc.tensor.matmul(pso,
                                         ytb_all[:, hp, tt * 128:(tt + 1) * 128],
                                         wo_sb[:, hp, oc * 512:(oc + 1) * 512],
                                         start=(hp == 0), stop=(hp == NHP - 1),
                                         skip_group_check=True)
                    ob = scp.tile([128, 512], F32, tag="ob")
                    nc.vector.tensor_copy(out=ob, in_=pso)
                    nc.sync.dma_start(
                        out=out_d[tt * 128:(tt + 1) * 128,
                                  oc * 512:(oc + 1) * 512],
                        in_=ob)

    nc.compile()
    return nc


def make_core_inputs(x, cos, sin, Wq, Wk, Wv, Wo, B, T):
    """Host-side sharding. Returns list of 8 input dicts.

    Core c handles batch c//2, head-group c%2 (8 heads each).
    """
    x = np.asarray(x, np.float32)
    cosT = np.asarray(cos, np.float32).reshape(T, D).T      # [64, T]
    sinT = np.asarray(sin, np.float32).reshape(T, D).T
    # sin_signed (dest-row order) = [-sin[0:32], sin[32:64]]; the kernel
    # indexes sin by SOURCE row (rot-half swaps 32-blocks), so permute:
    # sin_perm[s] = sin_signed[d(s)] with d = 32-block swap.
    sin_perm = np.concatenate([sinT[32:64], -sinT[0:32]], axis=0)
    cos2 = np.ascontiguousarray(
        np.concatenate([cosT, cosT], axis=0)).astype(BF16)
    sin2 = np.ascontiguousarray(
        np.concatenate([sin_perm, sin_perm], axis=0)).astype(BF16)
    ones2c = np.zeros((128, 2), BF16)
    ones2c[0:64, 0] = 1.0
    ones2c[64:128, 1] = 1.0
    ident = np.eye(128, dtype=BF16)
    onesk = np.ones((128, T // 128), BF16)

    Wq = np.asarray(Wq, np.float32)
    Wk = np.asarray(Wk, np.float32)
    Wv = np.asarray(Wv, np.float32)
    Wo = np.asarray(Wo, np.float32)

    def tile_x(xb):
        # x[b].T [C, T] -> [ci=128, c=T//512, co=8, 512]
        xt = np.ascontiguousarray(xb.T).reshape(8, 128, T // 512, 512)
        return np.ascontiguousarray(xt.transpose(1, 2, 0, 3)).astype(BF16)

    def tile_w(w_rows):
        # W[rows].T [C, 512] -> [ci=128, co=8, hp=4, 128]
        wt = np.ascontiguousarray(w_rows.T).reshape(8, 128, 4, 128)
        return np.ascontiguousarray(wt.transpose(1, 0, 2, 3)).astype(BF16)

    def tile_wo(w_cols):
        # Wo[:, cols].T [512, C] -> [ci=128, hp=4, C]
        wt = np.ascontiguousarray(w_cols.T).reshape(4, 128, C)
        return np.ascontiguousarray(wt.transpose(1, 0, 2)).astype(BF16)

    xT_b = [tile_x(x[b]) for b in range(B)]
    wq_g = [tile_w(Wq[g * MG:(g + 1) * MG]) for g in range(2)]
    wk_g = [tile_w(Wk[g * MG:(g + 1) * MG]) for g in range(2)]
    wv_g = [tile_w(Wv[g * MG:(g + 1) * MG]) for g in range(2)]
    wo_g = [tile_wo(Wo[:, g * MG:(g + 1) * MG]) for g in range(2)]

    in_maps = []
    for core in range(N_CORES):
        b, g = core // 2, core % 2
        in_maps.append({
            "xT": xT_b[b],
            "wq": wq_g[g],
            "wk": wk_g[g],
            "wv": wv_g[g],
            "wo": wo_g[g],
            "cos2": cos2,
            "sin2s": sin2,
            "ident": ident,
            "ones2c": ones2c,
            "onesk": onesk,
        })
    return in_maps


def kernel(x, cos, sin, Wq, Wk, Wv, Wo):
    from concourse.bass_utils import run_bass_kernel_spmd

    B, T = x.shape[0], x.shape[1]
    key = T
    if key not in _NC_CACHE:
        _NC_CACHE[key] = build_nc(T)
    nc = _NC_CACHE[key]
    in_maps = make_core_inputs(x, cos, sin, Wq, Wk, Wv, Wo, B, T)
    res = run_bass_kernel_spmd(nc, in_maps, core_ids=list(range(N_CORES)))
    out = np.zeros((B, T, C), np.float32)
    for core, r in enumerate(res.results):
        out[core // 2] += r["out"]
    return out
